# revision 14
# baseline (speedup 1.0000x reference)
"""DKVMN scatter_memory kernel for 8 Trainium2 NeuronCores.

Math: the reference scan only ever uses the (B, M, Dv) memory through
read @ Wf_r, so the whole recurrence collapses to a 32-dim linear
cumulative sum:

  S  = softmax(Eq @ Wa + ba)            (100 x 32)  per-vocab att rows
  cq = Eq @ Wf[:64] + bf                (100,)
  cv = Ev @ Wf[64:]                     (100,)
  w  = (2q + a) % 100
  pred[t,b] = cq[q[t,b]] + sum_{s<t} cv[w[s,b]] * <S[q[t,b]], S[q[s,b]]>

Per core (batch-sharded, Bs=128): the host precomputes fp8 one-hot
encodings of q and w (pure index preprocessing; 0/1 are exact in fp8);
the device gathers S-rows / cq / cv by one-hot matmuls on TensorE
([S|cq] streamed in one 33-col matmul per batch element), then the
cumsum over t is a strict-lower-triangular matmul.  Layout: t on
partitions, (b, m) on free dim.  All parameters arrive in a single
packed f16 tensor; one-hots stream in group-blocked chunks across the
three DMA-capable queues (sync / scalar / gpsimd).
"""
import functools
import numpy as np
import ml_dtypes

import concourse.bass as bass
import concourse.bacc as bacc
import concourse.mybir as mybir
from concourse import tile
from concourse.bass_utils import run_bass_kernel_spmd

T, B, M, DQ, DV, VOCAB = 128, 1024, 32, 64, 64, 100
NCORES = 8
BS = B // NCORES  # 128
N = T * BS        # tokens per core = 16384
GB = 15           # b per group (33*15=495, +15 cv cols = 510 <= 512 psum bank)
GROUPS = [(g * GB, GB) for g in range(8)] + [(120, 8)]
GCOL = [2 * b0 * T for b0, _ in GROUPS]  # ohall column offset of each group
F32 = mybir.dt.float32
F16 = mybir.dt.float16
FP8 = mybir.dt.float8e4
AX = mybir.AxisListType
OP = mybir.AluOpType

# packed-parameter column layout (f16 [128, PC])
_EQ, _EV, _WA, _WFQ, _WFR = 0, 64, 128, 160, 161
_ID, _US, _ONE, _BA, _BF = 162, 262, 390, 490, 522
PC = 523

# one-hot chunk schedule: (queue, [group indices])
CHUNKS = [
    ("sync", [0, 1]),
    ("scalar", [2, 3]),
    ("sync", [4, 5, 6]),
    ("scalar", [7, 8]),
]


def _build():
    nc = bacc.Bacc("TRN2", num_devices=NCORES, debug=False, target_bir_lowering=False)
    d = {}
    d["pack"] = nc.dram_tensor("pack", [128, PC], F16, kind="ExternalInput").ap()
    d["ohall"] = nc.dram_tensor("ohall", [VOCAB, 2 * N], FP8, kind="ExternalInput").ap()
    preds = nc.dram_tensor("preds", [T, BS], F32, kind="ExternalOutput").ap()

    with tile.TileContext(nc) as tc:
        with (
            tc.tile_pool(name="sb", bufs=1) as sb,
            tc.tile_pool(name="wk", bufs=3) as wk,
            tc.tile_pool(name="ps", bufs=3, space="PSUM") as ps,
        ):
            P = sb.tile([128, PC], F16)
            nc.scalar.dma_start(P[:], d["pack"][:])
            gtile = [None] * len(GROUPS)
            goff = [0] * len(GROUPS)
            for qname, gids in CHUNKS:
                eng = getattr(nc, qname)
                c0 = GCOL[gids[0]]
                g_end = gids[-1]
                c1 = GCOL[g_end] + 2 * GROUPS[g_end][1] * T
                t_ = sb.tile([VOCAB, c1 - c0], FP8)
                eng.dma_start(t_[:], d["ohall"][:, c0:c1])
                for g in gids:
                    gtile[g] = t_
                    goff[g] = GCOL[g] - c0

            us_t = P[:, _US:_US + 128]

            # ---- parameter tables ----
            p_eqT = ps.tile([DQ, VOCAB], F16, tag="pA")
            p_evT = ps.tile([DV, VOCAB], F16, tag="pP")
            eqT_t = sb.tile([DQ, VOCAB], F16)
            evT_t = sb.tile([DV, VOCAB], F16)
            nc.tensor.transpose(p_eqT[:], P[0:VOCAB, _EQ:_EQ + DQ], P[0:VOCAB, _ID:_ID + VOCAB])
            nc.scalar.copy(eqT_t[:], p_eqT[:])
            nc.tensor.transpose(p_evT[:], P[0:VOCAB, _EV:_EV + DV], P[0:VOCAB, _ID:_ID + VOCAB])
            nc.scalar.copy(evT_t[:], p_evT[:])

            # scat = [softmax(Eq@Wa+ba) | cq]  (100 x 33) fp16
            scat = sb.tile([VOCAB, M + 1], F16)
            p_s = ps.tile([VOCAB, M], F32, tag="pA")
            nc.tensor.matmul(p_s[:], eqT_t[:], P[0:DQ, _WA:_WA + M], start=True, stop=False)
            nc.tensor.matmul(p_s[:], P[0:1, _ONE:_ONE + VOCAB], P[0:1, _BA:_BA + M],
                             start=False, stop=True)
            mx_t = sb.tile([VOCAB, 1], F32)
            sm_t = sb.tile([VOCAB, 1], F32)
            se_t = sb.tile([VOCAB, M], F32)
            nc.vector.tensor_reduce(mx_t[:], p_s[:], AX.X, OP.max)
            nc.vector.tensor_scalar_mul(mx_t[:], mx_t[:], -1.0)
            nc.scalar.activation(se_t[:], p_s[:],
                                 mybir.ActivationFunctionType.Exp,
                                 bias=mx_t[:], scale=1.0)
            nc.vector.tensor_reduce(sm_t[:], se_t[:], AX.X, OP.add)
            nc.vector.reciprocal(sm_t[:], sm_t[:])
            nc.vector.tensor_scalar(out=scat[:, 0:M], in0=se_t[:], scalar1=sm_t[:],
                                    scalar2=None, op0=OP.mult)
            p_cq = ps.tile([VOCAB, 1], F32, tag="pP")
            nc.tensor.matmul(p_cq[:], eqT_t[:], P[0:DQ, _WFQ:_WFQ + 1], start=True, stop=False)
            nc.tensor.matmul(p_cq[:], P[0:1, _ONE:_ONE + VOCAB], P[0:1, _BF:_BF + 1],
                             start=False, stop=True)
            nc.scalar.copy(scat[:, M:M + 1], p_cq[:])

            # cv = Ev @ Wf_r  (100 x 1) fp16
            cv16 = sb.tile([VOCAB, 1], F16)
            p_cv = ps.tile([VOCAB, 1], F32, tag="pA")
            nc.tensor.matmul(p_cv[:], evT_t[:], P[0:DV, _WFR:_WFR + 1], start=True, stop=True)
            nc.scalar.copy(cv16[:], p_cv[:])

            # ---- main pipeline ----
            out_sb = sb.tile([128, BS], F32)
            c_sb = sb.tile([128, BS], F32)

            for gi, (b0, gb) in enumerate(GROUPS):
                oh_g = gtile[gi]
                off = goff[gi]
                woff = off + gb * T
                pA = ps.tile([128, 510], F32, tag="pA")
                pP = ps.tile([128, 480], F32, tag="pP")
                a_g = wk.tile([128, 480], F16, tag="a_sb")
                v_g = wk.tile([128, 480], F16, tag="v_sb")
                ap_g = wk.tile([128, 480], F16, tag="ap_sb")
                cvw_g = wk.tile([128, GB], F16, tag="cvw_sb")
                for k in range(gb):
                    nc.tensor.matmul(pA[:, k * 33:k * 33 + 33],
                                     oh_g[:, off + k * T:off + (k + 1) * T], scat[:],
                                     start=True, stop=True)
                    nc.tensor.matmul(pA[:, 495 + k:496 + k],
                                     oh_g[:, woff + k * T:woff + (k + 1) * T], cv16[:],
                                     start=True, stop=True)
                pA3 = pA[:, 0:gb * 33].rearrange("p (k c) -> p k c", c=33)
                # cq column -> c_sb ; cv row -> f16 ; A rows -> f16
                nc.scalar.copy(c_sb[:, b0:b0 + gb], pA3[:, :, M:M + 1])
                nc.scalar.copy(cvw_g[:, 0:gb], pA[:, 495:495 + gb])
                nc.scalar.copy(a_g[:, 0:gb * M].rearrange("p (k c) -> p k c", c=M),
                               pA3[:, :, 0:M])
                # v = A * cv[w] (broadcast cvw along m)
                a3 = a_g[:, 0:gb * M].rearrange("p (k c) -> p k c", c=M)
                cvb = cvw_g[:, 0:gb].rearrange("p (k c) -> p k c", c=1)
                a3b, cvb = bass.broadcast_tensor_aps(a3, cvb)
                nc.vector.tensor_tensor(
                    v_g[:, 0:gb * M].rearrange("p (k c) -> p k c", c=M),
                    a3b, cvb, OP.mult)
                # exclusive cumsum over t (strict upper as lhsT)
                nc.tensor.matmul(pP[:, 0:gb * M], us_t, v_g[:, 0:gb * M],
                                 start=True, stop=True)
                # pred contribution: sum_m A * C
                nc.vector.tensor_tensor(
                    ap_g[:, 0:gb * M], a_g[:, 0:gb * M], pP[:, 0:gb * M], OP.mult)
                nc.vector.tensor_reduce(
                    out_sb[:, b0:b0 + gb],
                    ap_g[:, 0:gb * M].rearrange("p (b m) -> p b m", m=M),
                    AX.X, OP.add)

            nc.vector.tensor_add(out_sb[:], out_sb[:], c_sb[:])
            nc.sync.dma_start(preds[:], out_sb[:])

    nc.compile()
    return nc


@functools.lru_cache(maxsize=1)
def _get_nc():
    return _build()


def _in_maps(questions, answers, Eq, Ev, Wa, ba, Wf, bf):
    questions = np.asarray(questions)
    answers = np.asarray(answers)
    w = (questions.astype(np.int64) * 2 + answers.astype(np.int64)) % VOCAB
    pack = np.zeros((128, PC), np.float16)
    pack[0:VOCAB, _EQ:_EQ + DQ] = np.asarray(Eq, np.float32)
    pack[0:VOCAB, _EV:_EV + DV] = np.asarray(Ev, np.float32)
    pack[0:DQ, _WA:_WA + M] = np.asarray(Wa, np.float32)
    wf = np.asarray(Wf, np.float32).reshape(DQ + DV)
    pack[0:DQ, _WFQ] = wf[0:DQ]
    pack[0:DV, _WFR] = wf[DQ:DQ + DV]
    pack[0:VOCAB, _ID:_ID + VOCAB] = np.eye(VOCAB, dtype=np.float16)
    pack[:, _US:_US + 128] = np.triu(np.ones((128, 128), np.float16), k=1)
    pack[0, _ONE:_ONE + VOCAB] = 1.0
    pack[0, _BA:_BA + M] = np.asarray(ba, np.float32).reshape(M)
    pack[0, _BF] = np.asarray(bf, np.float32).reshape(())
    in_maps = []
    ar = np.arange(BS * T)
    for c in range(NCORES):
        sl = slice(c * BS, (c + 1) * BS)
        qf = np.ascontiguousarray(questions[:, sl].T).ravel()
        wfl = np.ascontiguousarray(w[:, sl].T).ravel()
        # group-blocked: [q cols | w cols] per group of GB batch elements
        oh = np.zeros((VOCAB, 2 * N), dtype=ml_dtypes.float8_e4m3)
        for gi, (b0, gb) in enumerate(GROUPS):
            base = GCOL[gi]
            tok = slice(b0 * T, (b0 + gb) * T)
            nt = (tok.stop - tok.start)
            oh[qf[tok], base + np.arange(nt)] = 1.0
            oh[wfl[tok], base + nt + np.arange(nt)] = 1.0
        in_maps.append({"pack": pack, "ohall": oh})
    return in_maps


def kernel(questions, answers, Eq, Ev, Wa, ba, Wf, bf):
    nc = _get_nc()
    in_maps = _in_maps(questions, answers, Eq, Ev, Wa, ba, Wf, bf)
    res = run_bass_kernel_spmd(nc, in_maps, list(range(NCORES)))
    preds = np.concatenate([res.results[c]["preds"] for c in range(NCORES)], axis=1)
    return preds.astype(np.float32)


# revision 15
# speedup vs baseline: 1.2855x; 1.2855x over previous
"""DKVMN scatter_memory kernel for 8 Trainium2 NeuronCores.

Math: the reference scan only ever uses the (B, M, Dv) memory through
read @ Wf_r, so the whole recurrence collapses to a 32-dim linear
cumulative sum:

  S  = softmax(Eq @ Wa + ba)            (100 x 32)  per-vocab att rows
  cq = Eq @ Wf[:64] + bf                (100,)
  cv = Ev @ Wf[64:]                     (100,)
  w  = (2q + a) % 100
  pred[t,b] = cq[q[t,b]] + sum_{s<t} cv[w[s,b]] * <S[q[t,b]], S[q[s,b]]>

Per core (batch-sharded, Bs=128): the host precomputes fp8 one-hot
encodings of q and w (pure index preprocessing; 0/1 are exact in fp8);
the device gathers S-rows / cq / cv by one-hot matmuls on TensorE
([S|cq] streamed in one 33-col matmul per batch element), then the
cumsum over t is a strict-lower-triangular matmul.  Layout: t on
partitions, (b, m) on free dim.  All parameters arrive in a single
packed f16 tensor; one-hots stream in group-blocked chunks across the
three DMA-capable queues (sync / scalar / gpsimd).
"""
import functools
import numpy as np
import ml_dtypes

import concourse.bass as bass
import concourse.bacc as bacc
import concourse.mybir as mybir
from concourse import tile
from concourse.bass_utils import run_bass_kernel_spmd

T, B, M, DQ, DV, VOCAB = 128, 1024, 32, 64, 64, 100
NCORES = 8
BS = B // NCORES  # 128
N = T * BS        # tokens per core = 16384
GB = 15           # b per group (33*15=495, +15 cv cols = 510 <= 512 psum bank)
GROUPS = [(g * GB, GB) for g in range(8)] + [(120, 8)]
GCOL = [2 * b0 * T for b0, _ in GROUPS]  # ohall column offset of each group
F32 = mybir.dt.float32
F16 = mybir.dt.float16
FP8 = mybir.dt.float8e4
AX = mybir.AxisListType
OP = mybir.AluOpType

# packed-parameter column layout (f16 [128, PC])
_EQ, _EV, _WA, _WFQ, _WFR = 0, 64, 128, 160, 161
_ID, _US, _ONE, _BA, _BF = 162, 262, 390, 490, 522
PC = 523

# one-hot chunk schedule: (queue, [group indices])
CHUNKS = [
    ("sync", [0, 1]),
    ("scalar", [2, 3]),
    ("sync", [4, 5, 6]),
    ("scalar", [7, 8]),
]


def _build():
    nc = bacc.Bacc("TRN2", num_devices=NCORES, debug=False, target_bir_lowering=False)
    d = {}
    d["pack"] = nc.dram_tensor("pack", [128, PC], F16, kind="ExternalInput").ap()
    d["ohall"] = nc.dram_tensor("ohall", [VOCAB, 2 * N], FP8, kind="ExternalInput").ap()
    preds = nc.dram_tensor("preds", [T, BS], F32, kind="ExternalOutput").ap()

    with tile.TileContext(nc) as tc:
        with (
            tc.tile_pool(name="sb", bufs=1) as sb,
            tc.tile_pool(name="wk", bufs=3) as wk,
            tc.tile_pool(name="ps", bufs=3, space="PSUM") as ps,
        ):
            P = sb.tile([128, PC], F16)
            nc.scalar.dma_start(P[:], d["pack"][:])
            gtile = [None] * len(GROUPS)
            goff = [0] * len(GROUPS)
            for ci, (qname, gids) in enumerate(CHUNKS):
                eng = getattr(nc, qname)
                c0 = GCOL[gids[0]]
                g_end = gids[-1]
                c1 = GCOL[g_end] + 2 * GROUPS[g_end][1] * T
                t_ = sb.tile([VOCAB, c1 - c0], FP8, name=f"oh_chunk_{ci}")
                eng.dma_start(t_[:], d["ohall"][:, c0:c1])
                for g in gids:
                    gtile[g] = t_
                    goff[g] = GCOL[g] - c0

            us_t = P[:, _US:_US + 128]

            # ---- parameter tables ----
            p_eqT = ps.tile([DQ, VOCAB], F16, tag="pA")
            p_evT = ps.tile([DV, VOCAB], F16, tag="pP")
            eqT_t = sb.tile([DQ, VOCAB], F16)
            evT_t = sb.tile([DV, VOCAB], F16)
            nc.tensor.transpose(p_eqT[:], P[0:VOCAB, _EQ:_EQ + DQ], P[0:VOCAB, _ID:_ID + VOCAB])
            nc.scalar.copy(eqT_t[:], p_eqT[:])
            nc.tensor.transpose(p_evT[:], P[0:VOCAB, _EV:_EV + DV], P[0:VOCAB, _ID:_ID + VOCAB])
            nc.scalar.copy(evT_t[:], p_evT[:])

            # scat = [softmax(Eq@Wa+ba) | cq]  (100 x 33) fp16
            scat = sb.tile([VOCAB, M + 1], F16)
            p_s = ps.tile([VOCAB, M], F32, tag="pA")
            nc.tensor.matmul(p_s[:], eqT_t[:], P[0:DQ, _WA:_WA + M], start=True, stop=False)
            nc.tensor.matmul(p_s[:], P[0:1, _ONE:_ONE + VOCAB], P[0:1, _BA:_BA + M],
                             start=False, stop=True)
            mx_t = sb.tile([VOCAB, 1], F32)
            sm_t = sb.tile([VOCAB, 1], F32)
            se_t = sb.tile([VOCAB, M], F32)
            nc.vector.tensor_reduce(mx_t[:], p_s[:], AX.X, OP.max)
            nc.vector.tensor_scalar_mul(mx_t[:], mx_t[:], -1.0)
            nc.scalar.activation(se_t[:], p_s[:],
                                 mybir.ActivationFunctionType.Exp,
                                 bias=mx_t[:], scale=1.0)
            nc.vector.tensor_reduce(sm_t[:], se_t[:], AX.X, OP.add)
            nc.vector.reciprocal(sm_t[:], sm_t[:])
            nc.vector.tensor_scalar(out=scat[:, 0:M], in0=se_t[:], scalar1=sm_t[:],
                                    scalar2=None, op0=OP.mult)
            p_cq = ps.tile([VOCAB, 1], F32, tag="pP")
            nc.tensor.matmul(p_cq[:], eqT_t[:], P[0:DQ, _WFQ:_WFQ + 1], start=True, stop=False)
            nc.tensor.matmul(p_cq[:], P[0:1, _ONE:_ONE + VOCAB], P[0:1, _BF:_BF + 1],
                             start=False, stop=True)
            nc.scalar.copy(scat[:, M:M + 1], p_cq[:])

            # cv = Ev @ Wf_r  (100 x 1) fp16
            cv16 = sb.tile([VOCAB, 1], F16)
            p_cv = ps.tile([VOCAB, 1], F32, tag="pA")
            nc.tensor.matmul(p_cv[:], evT_t[:], P[0:DV, _WFR:_WFR + 1], start=True, stop=True)
            nc.scalar.copy(cv16[:], p_cv[:])

            # ---- main pipeline ----
            out_sb = sb.tile([128, BS], F32)
            c_sb = sb.tile([128, BS], F32)

            for gi, (b0, gb) in enumerate(GROUPS):
                oh_g = gtile[gi]
                off = goff[gi]
                woff = off + gb * T
                pA = ps.tile([128, 510], F32, tag="pA")
                pP = ps.tile([128, 480], F32, tag="pP")
                a_g = wk.tile([128, 480], F16, tag="a_sb")
                v_g = wk.tile([128, 480], F16, tag="v_sb")
                ap_g = wk.tile([128, 480], F16, tag="ap_sb")
                cvw_g = wk.tile([128, GB], F16, tag="cvw_sb")
                for k in range(gb):
                    nc.tensor.matmul(pA[:, k * 33:k * 33 + 33],
                                     oh_g[:, off + k * T:off + (k + 1) * T], scat[:],
                                     start=True, stop=True)
                    nc.tensor.matmul(pA[:, 495 + k:496 + k],
                                     oh_g[:, woff + k * T:woff + (k + 1) * T], cv16[:],
                                     start=True, stop=True)
                pA3 = pA[:, 0:gb * 33].rearrange("p (k c) -> p k c", c=33)
                # cq column -> c_sb ; cv row -> f16 ; A rows -> f16
                nc.scalar.copy(c_sb[:, b0:b0 + gb], pA3[:, :, M:M + 1])
                nc.scalar.copy(cvw_g[:, 0:gb], pA[:, 495:495 + gb])
                nc.scalar.copy(a_g[:, 0:gb * M].rearrange("p (k c) -> p k c", c=M),
                               pA3[:, :, 0:M])
                # v = A * cv[w] (broadcast cvw along m)
                a3 = a_g[:, 0:gb * M].rearrange("p (k c) -> p k c", c=M)
                cvb = cvw_g[:, 0:gb].rearrange("p (k c) -> p k c", c=1)
                a3b, cvb = bass.broadcast_tensor_aps(a3, cvb)
                nc.vector.tensor_tensor(
                    v_g[:, 0:gb * M].rearrange("p (k c) -> p k c", c=M),
                    a3b, cvb, OP.mult)
                # exclusive cumsum over t (strict upper as lhsT)
                nc.tensor.matmul(pP[:, 0:gb * M], us_t, v_g[:, 0:gb * M],
                                 start=True, stop=True)
                # pred contribution: sum_m A * C
                nc.vector.tensor_tensor(
                    ap_g[:, 0:gb * M], a_g[:, 0:gb * M], pP[:, 0:gb * M], OP.mult)
                nc.vector.tensor_reduce(
                    out_sb[:, b0:b0 + gb],
                    ap_g[:, 0:gb * M].rearrange("p (b m) -> p b m", m=M),
                    AX.X, OP.add)

            nc.vector.tensor_add(out_sb[:], out_sb[:], c_sb[:])
            nc.sync.dma_start(preds[:], out_sb[:])

    nc.compile()
    return nc


@functools.lru_cache(maxsize=1)
def _get_nc():
    return _build()


def _in_maps(questions, answers, Eq, Ev, Wa, ba, Wf, bf):
    questions = np.asarray(questions)
    answers = np.asarray(answers)
    w = (questions.astype(np.int64) * 2 + answers.astype(np.int64)) % VOCAB
    pack = np.zeros((128, PC), np.float16)
    pack[0:VOCAB, _EQ:_EQ + DQ] = np.asarray(Eq, np.float32)
    pack[0:VOCAB, _EV:_EV + DV] = np.asarray(Ev, np.float32)
    pack[0:DQ, _WA:_WA + M] = np.asarray(Wa, np.float32)
    wf = np.asarray(Wf, np.float32).reshape(DQ + DV)
    pack[0:DQ, _WFQ] = wf[0:DQ]
    pack[0:DV, _WFR] = wf[DQ:DQ + DV]
    pack[0:VOCAB, _ID:_ID + VOCAB] = np.eye(VOCAB, dtype=np.float16)
    pack[:, _US:_US + 128] = np.triu(np.ones((128, 128), np.float16), k=1)
    pack[0, _ONE:_ONE + VOCAB] = 1.0
    pack[0, _BA:_BA + M] = np.asarray(ba, np.float32).reshape(M)
    pack[0, _BF] = np.asarray(bf, np.float32).reshape(())
    in_maps = []
    ar = np.arange(BS * T)
    for c in range(NCORES):
        sl = slice(c * BS, (c + 1) * BS)
        qf = np.ascontiguousarray(questions[:, sl].T).ravel()
        wfl = np.ascontiguousarray(w[:, sl].T).ravel()
        # group-blocked: [q cols | w cols] per group of GB batch elements
        oh = np.zeros((VOCAB, 2 * N), dtype=ml_dtypes.float8_e4m3)
        for gi, (b0, gb) in enumerate(GROUPS):
            base = GCOL[gi]
            tok = slice(b0 * T, (b0 + gb) * T)
            nt = (tok.stop - tok.start)
            oh[qf[tok], base + np.arange(nt)] = 1.0
            oh[wfl[tok], base + nt + np.arange(nt)] = 1.0
        in_maps.append({"pack": pack, "ohall": oh})
    return in_maps


def kernel(questions, answers, Eq, Ev, Wa, ba, Wf, bf):
    nc = _get_nc()
    in_maps = _in_maps(questions, answers, Eq, Ev, Wa, ba, Wf, bf)
    res = run_bass_kernel_spmd(nc, in_maps, list(range(NCORES)))
    preds = np.concatenate([res.results[c]["preds"] for c in range(NCORES)], axis=1)
    return preds.astype(np.float32)
